# revision 21
# baseline (speedup 1.0000x reference)
"""Trainium2 Bass kernel for nn_Chemical_feature_interaction.

Math (per sample b):
    u = x1 @ var_1.T                  # [B, 32]
    v = x2 @ var_2                    # [B, 32]
    x3[b, c*32+r] = v[b,c] * u[b,r]   # [B, 1024]
    out = MLP(x3)  (1024->512->128->32->1, ReLU between, bias everywhere)

Dataflow on device (per core, feature-major activations):
  - batch is sharded 8 ways (4096 rows/core), processed in chunks of 512
    columns (batch lives on the matmul free dim).
  - x1/x2 chunk tiles are loaded transposed ([128 feat, 512 batch]) via
    HWDGE DMA-transpose (fp16).
  - U4 = tile(u, 4) [128, 512] is computed directly by a matmul against a
    host-precomputed var1_rep = tile(var_1.T, (1,4)) stationary operand.
  - V_k [128, 512] (v broadcast 32x along partitions) is computed with a
    tiny selector matmul (K=32) per 128-feature chunk k of x3.
  - x3[:, k, :] = U4 * V_k on the vector engine.
  - MLP layers run feature-major: lhsT = W chunk [128, m<=128], rhs =
    activations [128, 512]; PSUM accumulate over K; ReLU+bias fused into
    the PSUM->SBUF copy on the scalar engine.
"""

import numpy as np

import concourse.bacc as bacc
import concourse.mybir as mybir
import concourse.tile as tile
from concourse import bass_utils

B = 32768
D = 1024
R = 32
NCORES = 8
BL = B // NCORES  # rows per core
NB = 512  # batch columns per chunk
NCHUNK = BL // NB
KC = D // 128  # k-chunks of the 1024 contraction dims

F16 = mybir.dt.float16
F32 = mybir.dt.float32
AF = mybir.ActivationFunctionType


def _build(b4_val: float, repeat: int = 1, variant: str = "full"):
    """Build the per-core Bass program. repeat>1 wraps the whole kernel in a
    device-side loop (benchmarking only — lets wall-clock differencing
    extract device time through the axon RPC overhead). variant selects
    ablated builds for bottleneck measurement (NOT numerically correct):
    "full" | "nopack" (vk without tile_position packing) | "nox3" (W1 reads
    u4 instead of x3; drops DVE+vk) | "wonly" (MLP only, x3:=x1t)."""
    nc = bacc.Bacc("TRN2", target_bir_lowering=False, debug=False)

    # x1/x2 arrive host-pre-transposed and chunk-blocked:
    # [NCHUNK, 128, KC, NB], [i, p, k, b] = x[i*NB + b, 128k + p] — each
    # chunk's tile is one fully contiguous 1 MB block.
    x1_d = nc.dram_tensor("x1", (NCHUNK, 128, KC, NB), F16,
                          kind="ExternalInput").ap()
    x2_d = nc.dram_tensor("x2", (NCHUNK, 128, KC, NB), F16,
                          kind="ExternalInput").ap()
    v1r_d = nc.dram_tensor("v1r", (128, KC, 128), F16, kind="ExternalInput").ap()
    v2_d = nc.dram_tensor("v2l", (128, KC, 128), F16, kind="ExternalInput").ap()
    sel_d = nc.dram_tensor("sel", (128, KC, 128), F16, kind="ExternalInput").ap()
    w1_d = nc.dram_tensor("w1l", (128, KC, 512), F16, kind="ExternalInput").ap()
    w2_d = nc.dram_tensor("w2l", (128, 4, 128), F16, kind="ExternalInput").ap()
    w3_d = nc.dram_tensor("w3l", (128, 128), F16, kind="ExternalInput").ap()
    w4_d = nc.dram_tensor("w4l", (128, 1), F16, kind="ExternalInput").ap()
    b1_d = nc.dram_tensor("b1l", (128, 4), F32, kind="ExternalInput").ap()
    b2_d = nc.dram_tensor("b2l", (128, 1), F32, kind="ExternalInput").ap()
    b3_d = nc.dram_tensor("b3l", (128, 1), F32, kind="ExternalInput").ap()
    out_d = nc.dram_tensor("out", (BL, 1), F32, kind="ExternalOutput").ap()

    with tile.TileContext(nc) as tc:
        with (
            tc.tile_pool(name="consts", bufs=1) as consts,
            tc.tile_pool(name="io", bufs=3) as io,
            tc.tile_pool(name="work", bufs=2) as work,
            tc.tile_pool(name="psum", bufs=2, space="PSUM") as psum,
        ):
            v1r_sb = consts.tile([128, KC, 128], F16)
            v2_sb = consts.tile([128, KC, 128], F16)
            sel_sb = consts.tile([128, KC, 128], F16)
            w1_sb = consts.tile([128, KC, 512], F16)
            w2_sb = consts.tile([128, 4, 128], F16)
            w3_sb = consts.tile([128, 128], F16)
            w4_sb = consts.tile([128, 1], F16)
            b1_sb = consts.tile([128, 4], F32)
            b2_sb = consts.tile([128, 1], F32)
            b3_sb = consts.tile([128, 1], F32)
            for sb, dr in (
                (v1r_sb, v1r_d), (v2_sb, v2_d), (sel_sb, sel_d),
                (w1_sb, w1_d), (w2_sb, w2_d), (w3_sb, w3_d), (w4_sb, w4_d),
                (b1_sb, b1_d), (b2_sb, b2_d), (b3_sb, b3_d),
            ):
                nc.sync.dma_start(sb[:], dr[:])

            def body():
                _chunks(nc, x1_d, x2_d, out_d, v1r_sb, v2_sb, sel_sb, w1_sb,
                        w2_sb, w3_sb, w4_sb, b1_sb, b2_sb, b3_sb, b4_val,
                        io, work, psum, variant)

            if repeat == 1:
                body()
            else:
                with tc.For_i(0, repeat, 1):
                    body()

    nc.compile()
    return nc


def _chunks(nc, x1_d, x2_d, out_d, v1r_sb, v2_sb, sel_sb, w1_sb, w2_sb,
            w3_sb, w4_sb, b1_sb, b2_sb, b3_sb, b4_val, io, work, psum,
            variant="full"):
    if True:
        if True:  # keep original indentation of the chunk loop
            for i in range(NCHUNK):
                bs = slice(i * NB, (i + 1) * NB)

                x1t = io.tile([128, KC, NB], F16, tag="x1t")
                x2t = io.tile([128, KC, NB], F16, tag="x2t")
                if i == 0:
                    # k-sliced loads so the first u4/v matmuls start ~1/8 of
                    # the way into the transfer instead of after all 2 MB.
                    for k in range(KC):
                        nc.sync.dma_start(x1t[:, k, :], x1_d[i, :, k, :])
                        nc.scalar.dma_start(x2t[:, k, :], x2_d[i, :, k, :])
                else:
                    # x1 on the SP HWDGE ring, x2 on the ACT HWDGE ring
                    nc.sync.dma_start(x1t[:], x1_d[i])
                    nc.scalar.dma_start(x2t[:], x2_d[i])

                if variant == "wonly":
                    x3 = x1t
                else:
                    u4_ps = psum.tile([128, NB], F32, tag="uv")
                    v_ps = psum.tile([128, NB], F32, tag="uv")
                    for k in range(KC):
                        nc.tensor.matmul(u4_ps, v1r_sb[:, k, :], x1t[:, k, :],
                                         start=(k == 0), stop=(k == KC - 1))
                    for k in range(KC):
                        nc.tensor.matmul(v_ps, v2_sb[:, k, :], x2t[:, k, :],
                                         start=(k == 0), stop=(k == KC - 1))

                    u4_sb = work.tile([128, NB], F16, tag="u4")
                    v_sb = work.tile([128, NB], F16, tag="vsb")
                    nc.vector.tensor_copy(u4_sb, u4_ps)  # DVE ‖ ACT
                    nc.scalar.activation(v_sb, v_ps, AF.Copy)

                if variant in ("full", "nopack"):
                    x3 = work.tile([128, KC, NB], F16, tag="x3")
                    for k in range(KC):
                        vk_ps = psum.tile([128, NB], F32, tag="vk", bufs=4)
                        nc.tensor.matmul(
                            vk_ps, sel_sb[0:32, k, :], v_sb[0:32, :],
                            start=True, stop=True)
                        nc.vector.tensor_mul(x3[:, k, :], vk_ps, u4_sb)
                elif variant == "nox3":
                    x3 = work.tile([128, KC, NB], F16, tag="x3")
                    for k in range(KC):
                        nc.vector.tensor_copy(x3[:, k, :], u4_sb)

                h1 = work.tile([128, 4, NB], F16, tag="h1")
                for m in range(4):
                    h1_ps = psum.tile([128, NB], F32, tag="h1ps")
                    for k in range(KC):
                        nc.tensor.matmul(
                            h1_ps, w1_sb[:, k, m * 128:(m + 1) * 128], x3[:, k, :],
                            start=(k == 0), stop=(k == KC - 1))
                    nc.scalar.activation(h1[:, m, :], h1_ps, AF.Relu,
                                         bias=b1_sb[:, m:m + 1])

                h2_ps = psum.tile([128, NB], F32, tag="h1ps")
                for k in range(4):
                    nc.tensor.matmul(h2_ps, w2_sb[:, k, :], h1[:, k, :],
                                     start=(k == 0), stop=(k == 3))
                h2 = work.tile([128, NB], F16, tag="h2")
                nc.scalar.activation(h2, h2_ps, AF.Relu, bias=b2_sb[:, 0:1])

                h3_ps = psum.tile([128, NB], F32, tag="h1ps")
                nc.tensor.matmul(h3_ps, w3_sb, h2, start=True, stop=True)
                h3 = work.tile([128, NB], F16, tag="h3")
                nc.scalar.activation(h3, h3_ps, AF.Relu, bias=b3_sb[:, 0:1])

                o_ps = psum.tile([1, NB], F32, tag="h1ps")
                nc.tensor.matmul(o_ps, w4_sb, h3, start=True, stop=True)
                o_sb = work.tile([1, NB], F32, tag="osb")
                nc.scalar.activation(o_sb, o_ps, AF.Copy, bias=b4_val)

                nc.sync.dma_start(out_d[bs, :].rearrange("a b -> b a"), o_sb)


def _prep_consts(var_1, var_2, W1, b1, W2, b2, W3, b3, W4):
    f16 = np.float16
    v1r = np.tile(var_1.T, (1, 4))  # [1024, 128] -> U4 = tile(u, 4)
    v1r_l = np.ascontiguousarray(
        v1r.reshape(KC, 128, 128).transpose(1, 0, 2)).astype(f16)
    v2r = np.tile(var_2, (1, 4))  # [1024, 128] -> V4 = tile(v, 4)
    v2_l = np.ascontiguousarray(
        v2r.reshape(KC, 128, 128).transpose(1, 0, 2)).astype(f16)

    # selector for the V_k partition-broadcast matmuls (K=32)
    sel = np.zeros((128, KC, 128), f16)
    p = np.arange(128)
    for k in range(KC):
        sel[4 * k + p // 32, k, p] = 1.0

    w1_l = np.ascontiguousarray(
        W1.reshape(KC, 128, 512).transpose(1, 0, 2)).astype(f16)
    w2_l = np.ascontiguousarray(
        W2.reshape(4, 128, 128).transpose(1, 0, 2)).astype(f16)
    w3_l = np.zeros((128, 128), f16)
    w3_l[:, :R] = W3.astype(f16)
    w4_l = np.zeros((128, 1), f16)
    w4_l[:R, :] = W4.astype(f16)

    b1_l = np.ascontiguousarray(b1.reshape(4, 128).T).astype(np.float32)
    b2_l = b2.reshape(128, 1).astype(np.float32)
    b3_l = np.zeros((128, 1), np.float32)
    b3_l[:R, 0] = b3
    return dict(v1r=v1r_l, v2l=v2_l, sel=sel, w1l=w1_l, w2l=w2_l,
                w3l=w3_l, w4l=w4_l, b1l=b1_l, b2l=b2_l, b3l=b3_l)


def make_in_maps(**inputs):
    """Shard inputs across cores; returns (in_maps, b4_val)."""
    x1 = np.asarray(inputs["x1"], np.float32)
    x2 = np.asarray(inputs["x2"], np.float32)
    consts = _prep_consts(
        *[np.asarray(inputs[k], np.float32) for k in
          ("var_1", "var_2", "W1", "b1", "W2", "b2", "W3", "b3", "W4")])
    x1h = x1.astype(np.float16)
    x2h = x2.astype(np.float16)

    def shard_t(xh, c):
        # [BL, D] -> [NCHUNK, 128, KC, NB]: [i, p, k, b] = x[i*NB+b, 128k+p]
        xs = xh[c * BL:(c + 1) * BL]  # [BL, D]
        x4 = xs.reshape(NCHUNK, NB, KC, 128)  # [i, b, k, p]
        return np.ascontiguousarray(x4.transpose(0, 3, 2, 1))

    in_maps = []
    for c in range(NCORES):
        m = dict(consts)
        m["x1"] = shard_t(x1h, c)
        m["x2"] = shard_t(x2h, c)
        in_maps.append(m)
    return in_maps, float(np.asarray(inputs["b4"]).reshape(-1)[0])


def run(trace=False, **inputs):
    in_maps, b4_val = make_in_maps(**inputs)
    nc = _build(b4_val)
    res = bass_utils.run_bass_kernel_spmd(
        nc, in_maps, core_ids=list(range(NCORES)), trace=trace)
    out = np.concatenate([r["out"] for r in res.results], axis=0)
    return out.astype(np.float32), res


def kernel(**inputs):
    out, _ = run(trace=False, **inputs)
    return out


# revision 38
# speedup vs baseline: 1.9482x; 1.9482x over previous
"""Trainium2 Bass kernel for nn_Chemical_feature_interaction.

Math (per sample b):
    u = x1 @ var_1.T                  # [B, 32]
    v = x2 @ var_2                    # [B, 32]
    x3[b, c*32+r] = v[b,c] * u[b,r]   # [B, 1024]
    out = MLP(x3)  (1024->512->128->32->1, ReLU between, bias everywhere)

Dataflow on device (per core, feature-major activations):
  - batch is sharded 8 ways (4096 rows/core), processed in chunks of 512
    columns (batch lives on the matmul free dim).
  - x1/x2 chunk tiles are loaded transposed ([128 feat, 512 batch]) via
    HWDGE DMA-transpose (fp16).
  - U4 = tile(u, 4) [128, 512] is computed directly by a matmul against a
    host-precomputed var1_rep = tile(var_1.T, (1,4)) stationary operand.
  - V_k [128, 512] (v broadcast 32x along partitions) is computed with a
    tiny selector matmul (K=32) per 128-feature chunk k of x3.
  - x3[:, k, :] = U4 * V_k on the vector engine.
  - MLP layers run feature-major: lhsT = W chunk [128, m<=128], rhs =
    activations [128, 512]; PSUM accumulate over K; ReLU+bias fused into
    the PSUM->SBUF copy on the scalar engine.
"""

import numpy as np

import concourse.bacc as bacc
import concourse.mybir as mybir
import concourse.tile as tile
from concourse import bass_utils

B = 32768
D = 1024
R = 32
NCORES = 8
BL = B // NCORES  # rows per core
NB = 512  # batch columns per chunk
NCHUNK = BL // NB
KC = D // 128  # k-chunks of the 1024 contraction dims

F16 = mybir.dt.float16
F32 = mybir.dt.float32
AF = mybir.ActivationFunctionType

# scheduling knobs (A/B-tested on hardware; see bench_ablate.py)
CONFIG = {
    "w1_order": "k",   # "k": k-outer/m-inner (4 h1 banks); "m": m-outer
    "h1_bufs": 4,
    "tail_tag": "h1ps",  # "h1ps": share W1 banks; "tail": own tag
    "tail_bufs": 4,
}


def _build(b4_val: float, repeat: int = 1, variant: str = "full"):
    """Build the per-core Bass program. repeat>1 wraps the whole kernel in a
    device-side loop (benchmarking only — lets wall-clock differencing
    extract device time through the axon RPC overhead). variant selects
    ablated builds for bottleneck measurement (NOT numerically correct):
    "full" | "nopack" (alias of full) | "nox3" (W1 reads u4 copies; drops
    vk matmuls) | "wonly" (MLP only, x3:=x1t) | "dmaonly" (loads + trivial
    consume) | "w1only" (loads + W1 stage, no tail)."""
    nc = bacc.Bacc("TRN2", target_bir_lowering=False, debug=False)

    # x1/x2 arrive host-pre-transposed and chunk-blocked:
    # [NCHUNK, 128, KC, NB], [i, p, k, b] = x[i*NB + b, 128k + p] — each
    # chunk's tile is one fully contiguous 1 MB block.
    x1_d = nc.dram_tensor("x1", (NCHUNK, 128, KC, NB), F16,
                          kind="ExternalInput").ap()
    x2_d = nc.dram_tensor("x2", (NCHUNK, 128, KC, NB), F16,
                          kind="ExternalInput").ap()
    v1r_d = nc.dram_tensor("v1r", (128, KC, 128), F16, kind="ExternalInput").ap()
    v2_d = nc.dram_tensor("v2l", (128, KC, 128), F16, kind="ExternalInput").ap()
    sel_d = nc.dram_tensor("sel", (128, KC, 128), F16, kind="ExternalInput").ap()
    w1_d = nc.dram_tensor("w1l", (128, KC, 512), F16, kind="ExternalInput").ap()
    w2_d = nc.dram_tensor("w2l", (128, 4, 128), F16, kind="ExternalInput").ap()
    w3_d = nc.dram_tensor("w3l", (128, 128), F16, kind="ExternalInput").ap()
    w4_d = nc.dram_tensor("w4l", (128, 1), F16, kind="ExternalInput").ap()
    b1_d = nc.dram_tensor("b1l", (128, 4), F32, kind="ExternalInput").ap()
    b2_d = nc.dram_tensor("b2l", (128, 1), F32, kind="ExternalInput").ap()
    b3_d = nc.dram_tensor("b3l", (128, 1), F32, kind="ExternalInput").ap()
    out_d = nc.dram_tensor("out", (BL, 1), F32, kind="ExternalOutput").ap()

    with tile.TileContext(nc) as tc:
        with (
            tc.tile_pool(name="consts", bufs=1) as consts,
            tc.tile_pool(name="io", bufs=4) as io,
            tc.tile_pool(name="work", bufs=2) as work,
            tc.tile_pool(name="psum", bufs=2, space="PSUM") as psum,
        ):
            v1r_sb = consts.tile([128, KC, 128], F16)
            v2_sb = consts.tile([128, KC, 128], F16)
            sel_sb = consts.tile([128, KC, 128], F16)
            w1_sb = consts.tile([128, KC, 512], F16)
            w2_sb = consts.tile([128, 4, 128], F16)
            w3_sb = consts.tile([128, 128], F16)
            w4_sb = consts.tile([128, 1], F16)
            b1_sb = consts.tile([128, 4], F32)
            b2_sb = consts.tile([128, 1], F32)
            b3_sb = consts.tile([128, 1], F32)
            for sb, dr in (
                (v1r_sb, v1r_d), (v2_sb, v2_d), (sel_sb, sel_d),
                (w1_sb, w1_d), (w2_sb, w2_d), (w3_sb, w3_d), (w4_sb, w4_d),
                (b1_sb, b1_d), (b2_sb, b2_d), (b3_sb, b3_d),
            ):
                nc.sync.dma_start(sb[:], dr[:])

            def body():
                _chunks(nc, x1_d, x2_d, out_d, v1r_sb, v2_sb, sel_sb, w1_sb,
                        w2_sb, w3_sb, w4_sb, b1_sb, b2_sb, b3_sb, b4_val,
                        io, work, psum, variant)

            if repeat == 1:
                body()
            else:
                ET = mybir.EngineType
                with tc.For_i(0, repeat, 1,
                              hint_engines=(ET.PE, ET.Activation, ET.SP,
                                            ET.DVE, ET.Pool)):
                    body()

    nc.compile()
    return nc


def _chunks(nc, x1_d, x2_d, out_d, v1r_sb, v2_sb, sel_sb, w1_sb, w2_sb,
            w3_sb, w4_sb, b1_sb, b2_sb, b3_sb, b4_val, io, work, psum,
            variant="full"):
    """Emit the 8 batch chunks. With CONFIG["pipeline"], emission is
    software-pipelined: head(i+1) (loads, u/v matmuls, copies, vk+x3) is
    emitted before tail(i) (W1..W4 + activations + store), so every
    engine's FIFO queue interleaves the two chunks and the serial
    PE<->ACT ping-pong of chunk i's tail hides under chunk i+1's work."""

    def head(i):
        bs = slice(i * NB, (i + 1) * NB)
        x1t = io.tile([128, KC, NB], F16, tag="x1t")
        x2t = io.tile([128, KC, NB], F16, tag="x2t")
        if i == 0:
            # k-sliced loads so the first u4/v matmuls start ~1/8 of the
            # way into the transfer instead of after all 2 MB.
            for k in range(KC):
                nc.sync.dma_start(x1t[:, k, :], x1_d[i, :, k, :])
                nc.sync.dma_start(x2t[:, k, :], x2_d[i, :, k, :])
        else:
            nc.sync.dma_start(x1t[:], x1_d[i])
            nc.sync.dma_start(x2t[:], x2_d[i])

        if variant == "dmaonly":
            o_sb = work.tile([1, NB], F32, tag="osb")
            nc.vector.tensor_add(o_sb, x1t[0:1, 0, :], x2t[0:1, 0, :])
            nc.gpsimd.dma_start(out_d[bs, :].rearrange("a b -> b a"), o_sb)
            return None
        if variant in ("wonly", "w1only"):
            return x1t

        u4_ps = psum.tile([128, NB], F32, tag="uv")
        v_ps = psum.tile([128, NB], F32, tag="uv")
        for k in range(KC):
            nc.tensor.matmul(u4_ps, v1r_sb[:, k, :], x1t[:, k, :],
                             start=(k == 0), stop=(k == KC - 1))
        for k in range(KC):
            nc.tensor.matmul(v_ps, v2_sb[:, k, :], x2t[:, k, :],
                             start=(k == 0), stop=(k == KC - 1))

        u4_sb = work.tile([128, NB], F16, tag="u4")
        v_sb = work.tile([128, NB], F16, tag="vsb")
        if CONFIG.get("copies", "dve") == "dve":
            # keep ACT pure-Relu: function switches force ~1.3us ACT
            # function-table reloads that the cost model doesn't charge
            nc.vector.tensor_copy(v_sb, v_ps)
            nc.vector.tensor_copy(u4_sb, u4_ps)
        else:
            nc.vector.tensor_copy(u4_sb, u4_ps)  # DVE, parallel with ACT
            nc.scalar.activation(v_sb, v_ps, AF.Copy)

        x3 = work.tile([128, KC, NB], F16, tag="x3")
        for k in range(KC):
            if variant == "nox3":
                nc.vector.tensor_copy(x3[:, k, :], u4_sb)
                continue
            if CONFIG.get("vk", "mm") == "dma":
                # V_k via SBUF->SBUF broadcast DMA (stride-0 partition
                # reads) instead of a PE selector matmul.
                vk_sb = work.tile([128, NB], F16, tag="vkb", bufs=3)
                src = v_sb[4 * k:4 * k + 4, None, :].to_broadcast([4, 32, NB])
                dst = vk_sb.rearrange("(c q) n -> c q n", q=32)
                eng = getattr(nc, CONFIG.get("vk_dma_engine", "gpsimd"))
                eng.dma_start(dst, src)
                nc.vector.tensor_mul(x3[:, k, :], vk_sb, u4_sb)
            else:
                vk_ps = psum.tile([128, NB], F32,
                                  tag=CONFIG.get("vk_tag", "vk"),
                                  bufs=CONFIG.get("vk_bufs", 2))
                nc.tensor.matmul(vk_ps, sel_sb[0:32, k, :], v_sb[0:32, :],
                                 start=True, stop=True)
                nc.vector.tensor_mul(x3[:, k, :], vk_ps, u4_sb)
        return x3

    def tail(i, x3):
        bs = slice(i * NB, (i + 1) * NB)
        h1 = work.tile([128, 4, NB], F16, tag="h1")
        if CONFIG["w1_order"] == "k":
            # k-outer/m-inner: W1 matmuls for x3[k] start as soon as the
            # DVE produces that slice; 4 banks accumulate m=0..3.
            h1_pss = [psum.tile([128, NB], F32, tag="h1ps",
                                bufs=CONFIG["h1_bufs"],
                                name=f"h1ps{m}") for m in range(4)]
            for k in range(KC):
                for m in range(4):
                    nc.tensor.matmul(
                        h1_pss[m], w1_sb[:, k, m * 128:(m + 1) * 128],
                        x3[:, k, :],
                        start=(k == 0), stop=(k == KC - 1))
            for m in range(4):
                nc.scalar.activation(h1[:, m, :], h1_pss[m], AF.Relu,
                                     bias=b1_sb[:, m:m + 1])
        else:
            for m in range(4):
                h1_ps = psum.tile([128, NB], F32, tag="h1ps",
                                  bufs=CONFIG["h1_bufs"])
                for k in range(KC):
                    nc.tensor.matmul(
                        h1_ps, w1_sb[:, k, m * 128:(m + 1) * 128],
                        x3[:, k, :],
                        start=(k == 0), stop=(k == KC - 1))
                nc.scalar.activation(h1[:, m, :], h1_ps, AF.Relu,
                                     bias=b1_sb[:, m:m + 1])

        if variant == "w1only":
            o_sb = work.tile([1, NB], F32, tag="osb")
            nc.vector.tensor_copy(o_sb, h1[0:1, 0, :])
            nc.gpsimd.dma_start(out_d[bs, :].rearrange("a b -> b a"), o_sb)
            return

        tt, tb = CONFIG["tail_tag"], CONFIG["tail_bufs"]
        h2_ps = psum.tile([128, NB], F32, tag=tt, bufs=tb)
        for k in range(4):
            nc.tensor.matmul(h2_ps, w2_sb[:, k, :], h1[:, k, :],
                             start=(k == 0), stop=(k == 3))
        h2 = work.tile([128, NB], F16, tag="h2")
        nc.scalar.activation(h2, h2_ps, AF.Relu, bias=b2_sb[:, 0:1])

        h3_ps = psum.tile([128, NB], F32, tag=tt, bufs=tb)
        nc.tensor.matmul(h3_ps, w3_sb, h2, start=True, stop=True)
        h3 = work.tile([128, NB], F16, tag="h3")
        nc.scalar.activation(h3, h3_ps, AF.Relu, bias=b3_sb[:, 0:1])

        o_ps = psum.tile([1, NB], F32, tag=tt, bufs=tb)
        nc.tensor.matmul(o_ps, w4_sb, h3, start=True, stop=True)
        o_sb = work.tile([1, NB], F32, tag="osb")
        if CONFIG.get("copies", "dve") == "dve":
            nc.vector.tensor_scalar_add(o_sb, o_ps, b4_val)
        else:
            nc.scalar.activation(o_sb, o_ps, AF.Copy, bias=b4_val)

        # store via SWDGE: keeps the SP HWDGE ring pure input loads
        nc.gpsimd.dma_start(out_d[bs, :].rearrange("a b -> b a"), o_sb)

    if variant == "dmaonly":
        for i in range(NCHUNK):
            head(i)
        return

    if CONFIG.get("pipeline", True):
        prev = None
        for i in range(NCHUNK):
            x3 = head(i)
            if prev is not None:
                tail(i - 1, prev)
            prev = x3
        tail(NCHUNK - 1, prev)
    else:
        for i in range(NCHUNK):
            tail(i, head(i))


def _prep_consts(var_1, var_2, W1, b1, W2, b2, W3, b3, W4):
    f16 = np.float16
    v1r = np.tile(var_1.T, (1, 4))  # [1024, 128] -> U4 = tile(u, 4)
    v1r_l = np.ascontiguousarray(
        v1r.reshape(KC, 128, 128).transpose(1, 0, 2)).astype(f16)
    v2r = np.tile(var_2, (1, 4))  # [1024, 128] -> V4 = tile(v, 4)
    v2_l = np.ascontiguousarray(
        v2r.reshape(KC, 128, 128).transpose(1, 0, 2)).astype(f16)

    # selector for the V_k partition-broadcast matmuls (K=32)
    sel = np.zeros((128, KC, 128), f16)
    p = np.arange(128)
    for k in range(KC):
        sel[4 * k + p // 32, k, p] = 1.0

    w1_l = np.ascontiguousarray(
        W1.reshape(KC, 128, 512).transpose(1, 0, 2)).astype(f16)
    w2_l = np.ascontiguousarray(
        W2.reshape(4, 128, 128).transpose(1, 0, 2)).astype(f16)
    w3_l = np.zeros((128, 128), f16)
    w3_l[:, :R] = W3.astype(f16)
    w4_l = np.zeros((128, 1), f16)
    w4_l[:R, :] = W4.astype(f16)

    b1_l = np.ascontiguousarray(b1.reshape(4, 128).T).astype(np.float32)
    b2_l = b2.reshape(128, 1).astype(np.float32)
    b3_l = np.zeros((128, 1), np.float32)
    b3_l[:R, 0] = b3
    return dict(v1r=v1r_l, v2l=v2_l, sel=sel, w1l=w1_l, w2l=w2_l,
                w3l=w3_l, w4l=w4_l, b1l=b1_l, b2l=b2_l, b3l=b3_l)


def make_in_maps(**inputs):
    """Shard inputs across cores; returns (in_maps, b4_val)."""
    x1 = np.asarray(inputs["x1"], np.float32)
    x2 = np.asarray(inputs["x2"], np.float32)
    consts = _prep_consts(
        *[np.asarray(inputs[k], np.float32) for k in
          ("var_1", "var_2", "W1", "b1", "W2", "b2", "W3", "b3", "W4")])
    x1h = x1.astype(np.float16)
    x2h = x2.astype(np.float16)

    def shard_t(xh, c):
        # [BL, D] -> [NCHUNK, 128, KC, NB]: [i, p, k, b] = x[i*NB+b, 128k+p]
        xs = xh[c * BL:(c + 1) * BL]  # [BL, D]
        x4 = xs.reshape(NCHUNK, NB, KC, 128)  # [i, b, k, p]
        return np.ascontiguousarray(x4.transpose(0, 3, 2, 1))

    in_maps = []
    for c in range(NCORES):
        m = dict(consts)
        m["x1"] = shard_t(x1h, c)
        m["x2"] = shard_t(x2h, c)
        in_maps.append(m)
    return in_maps, float(np.asarray(inputs["b4"]).reshape(-1)[0])


def run(trace=False, **inputs):
    in_maps, b4_val = make_in_maps(**inputs)
    nc = _build(b4_val)
    res = bass_utils.run_bass_kernel_spmd(
        nc, in_maps, core_ids=list(range(NCORES)), trace=trace)
    out = np.concatenate([r["out"] for r in res.results], axis=0)
    return out.astype(np.float32), res


def kernel(**inputs):
    out, _ = run(trace=False, **inputs)
    return out


# revision 42
# speedup vs baseline: 2.0058x; 1.0296x over previous
"""Trainium2 Bass kernel for nn_Chemical_feature_interaction.

Math (per sample b):
    u = x1 @ var_1.T                  # [B, 32]
    v = x2 @ var_2                    # [B, 32]
    x3[b, c*32+r] = v[b,c] * u[b,r]   # [B, 1024]
    out = MLP(x3)  (1024->512->128->32->1, ReLU between, bias everywhere)

Dataflow on device (per core, feature-major activations, all matmul
operands fp16, fp32 PSUM accumulation; end-to-end rel err ~1.5e-3):
  - batch is sharded 8 ways (4096 rows/core), processed in chunks of 512
    columns (batch lives on the matmul free dim).
  - x1/x2 are pre-transposed AND chunk-blocked on the host so each chunk's
    [128 feat, 8 kchunks, 512 batch] tile is one contiguous 1 MB DMA.
  - U4 = tile(u, 4) [128, 512] comes directly out of a matmul against the
    host-precomputed var1_rep = tile(var_1.T, (1,4)) stationary operand
    (M=128 costs the same as M=32); same for V4 = tile(v, 4).
  - V_k [128, 512] (v[4k + p//32] per partition p) is produced by a tiny
    K=32 selector matmul per 128-feature chunk k (the PE is the only
    engine that can permute/broadcast across partitions at line rate).
  - x3[:, k, :] = U4 * V_k on the vector engine (PSUM x SBUF -> SBUF).
  - MLP layers run feature-major: lhsT = W chunk [128, m<=128], rhs =
    activations [128, 512]; PSUM accumulate over K; ReLU+bias fused into
    the PSUM->SBUF copy on the scalar engine.
  - Emission is software-pipelined (head(i+1) before tail(i)).

Measured (axon trn2, wall-clock differencing of a device-side repeat
loop): ~155-170 us per full pass; cost-model (TimelineSim) ~138 us; PE
busy ~113 us of that (the kernel is tensor-engine-bound).
"""

import numpy as np

import concourse.bacc as bacc
import concourse.mybir as mybir
import concourse.tile as tile
from concourse import bass_utils

B = 32768
D = 1024
R = 32
NCORES = 8
BL = B // NCORES  # rows per core
NB = 512  # batch columns per chunk
NCHUNK = BL // NB
KC = D // 128  # k-chunks of the 1024 contraction dims

F16 = mybir.dt.float16
F32 = mybir.dt.float32
AF = mybir.ActivationFunctionType

# scheduling knobs (A/B-tested on hardware; see bench_ablate.py)
CONFIG = {
    "w1_order": "k",   # "k": k-outer/m-inner (4 h1 banks); "m": m-outer
    "h1_bufs": 4,
    "tail_tag": "h1ps",  # "h1ps": share W1 banks; "tail": own tag
    "tail_bufs": 4,
    "pipeline": True,  # software-pipeline head(i+1) before tail(i)
    "copies": "dve",   # PSUM->SBUF copies on DVE (ACT stays pure-Relu)
    "vk": "mm",        # V_k broadcast via PE selector matmul ("dma" is
                       # broken: DMA collapses stride-0 source dims)
    "vk_tag": "vk",
    "vk_bufs": 2,
    "vk_dma_engine": "gpsimd",
}


def _build(b4_val: float, repeat: int = 1, variant: str = "full"):
    """Build the per-core Bass program. repeat>1 wraps the whole kernel in a
    device-side loop (benchmarking only — lets wall-clock differencing
    extract device time through the axon RPC overhead). variant selects
    ablated builds for bottleneck measurement (NOT numerically correct):
    "full" | "nopack" (alias of full) | "nox3" (W1 reads u4 copies; drops
    vk matmuls) | "wonly" (MLP only, x3:=x1t) | "dmaonly" (loads + trivial
    consume) | "w1only" (loads + W1 stage, no tail)."""
    nc = bacc.Bacc("TRN2", target_bir_lowering=False, debug=False)

    # x1/x2 arrive host-pre-transposed and chunk-blocked:
    # [NCHUNK, 128, KC, NB], [i, p, k, b] = x[i*NB + b, 128k + p] — each
    # chunk's tile is one fully contiguous 1 MB block.
    x1_d = nc.dram_tensor("x1", (NCHUNK, 128, KC, NB), F16,
                          kind="ExternalInput").ap()
    x2_d = nc.dram_tensor("x2", (NCHUNK, 128, KC, NB), F16,
                          kind="ExternalInput").ap()
    v1r_d = nc.dram_tensor("v1r", (128, KC, 128), F16, kind="ExternalInput").ap()
    v2_d = nc.dram_tensor("v2l", (128, KC, 128), F16, kind="ExternalInput").ap()
    sel_d = nc.dram_tensor("sel", (128, KC, 128), F16, kind="ExternalInput").ap()
    w1_d = nc.dram_tensor("w1l", (128, KC, 512), F16, kind="ExternalInput").ap()
    w2_d = nc.dram_tensor("w2l", (128, 4, 128), F16, kind="ExternalInput").ap()
    w3_d = nc.dram_tensor("w3l", (128, 128), F16, kind="ExternalInput").ap()
    w4_d = nc.dram_tensor("w4l", (128, 1), F16, kind="ExternalInput").ap()
    b1_d = nc.dram_tensor("b1l", (128, 4), F32, kind="ExternalInput").ap()
    b2_d = nc.dram_tensor("b2l", (128, 1), F32, kind="ExternalInput").ap()
    b3_d = nc.dram_tensor("b3l", (128, 1), F32, kind="ExternalInput").ap()
    out_d = nc.dram_tensor("out", (BL, 1), F32, kind="ExternalOutput").ap()

    with tile.TileContext(nc) as tc:
        with (
            tc.tile_pool(name="consts", bufs=1) as consts,
            tc.tile_pool(name="io", bufs=CONFIG.get("io_bufs", 4)) as io,
            tc.tile_pool(name="work", bufs=CONFIG.get("work_bufs", 2)) as work,
            tc.tile_pool(name="psum", bufs=2, space="PSUM") as psum,
        ):
            v1r_sb = consts.tile([128, KC, 128], F16)
            v2_sb = consts.tile([128, KC, 128], F16)
            sel_sb = consts.tile([128, KC, 128], F16)
            w1_sb = consts.tile([128, KC, 512], F16)
            w2_sb = consts.tile([128, 4, 128], F16)
            w3_sb = consts.tile([128, 128], F16)
            w4_sb = consts.tile([128, 1], F16)
            b1_sb = consts.tile([128, 4], F32)
            b2_sb = consts.tile([128, 1], F32)
            b3_sb = consts.tile([128, 1], F32)
            for sb, dr in (
                (v1r_sb, v1r_d), (v2_sb, v2_d), (sel_sb, sel_d),
                (w1_sb, w1_d), (w2_sb, w2_d), (w3_sb, w3_d), (w4_sb, w4_d),
                (b1_sb, b1_d), (b2_sb, b2_d), (b3_sb, b3_d),
            ):
                nc.sync.dma_start(sb[:], dr[:])

            def body():
                _chunks(nc, x1_d, x2_d, out_d, v1r_sb, v2_sb, sel_sb, w1_sb,
                        w2_sb, w3_sb, w4_sb, b1_sb, b2_sb, b3_sb, b4_val,
                        io, work, psum, variant)

            if repeat == 1:
                body()
            else:
                ET = mybir.EngineType
                with tc.For_i(0, repeat, 1,
                              hint_engines=(ET.PE, ET.Activation, ET.SP,
                                            ET.DVE, ET.Pool)):
                    body()

    nc.compile()
    return nc


def _chunks(nc, x1_d, x2_d, out_d, v1r_sb, v2_sb, sel_sb, w1_sb, w2_sb,
            w3_sb, w4_sb, b1_sb, b2_sb, b3_sb, b4_val, io, work, psum,
            variant="full"):
    """Emit the 8 batch chunks. With CONFIG["pipeline"], emission is
    software-pipelined: head(i+1) (loads, u/v matmuls, copies, vk+x3) is
    emitted before tail(i) (W1..W4 + activations + store), so every
    engine's FIFO queue interleaves the two chunks and the serial
    PE<->ACT ping-pong of chunk i's tail hides under chunk i+1's work."""

    def head(i):
        bs = slice(i * NB, (i + 1) * NB)
        x1t = io.tile([128, KC, NB], F16, tag="x1t")
        x2t = io.tile([128, KC, NB], F16, tag="x2t")
        if i == 0:
            # k-sliced loads so the first u4/v matmuls start ~1/8 of the
            # way into the transfer instead of after all 2 MB.
            for k in range(KC):
                nc.sync.dma_start(x1t[:, k, :], x1_d[i, :, k, :])
                nc.sync.dma_start(x2t[:, k, :], x2_d[i, :, k, :])
        else:
            nc.sync.dma_start(x1t[:], x1_d[i])
            nc.sync.dma_start(x2t[:], x2_d[i])

        if variant == "dmaonly":
            o_sb = work.tile([1, NB], F32, tag="osb")
            nc.vector.tensor_add(o_sb, x1t[0:1, 0, :], x2t[0:1, 0, :])
            nc.gpsimd.dma_start(out_d[bs, :].rearrange("a b -> b a"), o_sb)
            return None
        if variant in ("wonly", "w1only"):
            return x1t

        u4_ps = psum.tile([128, NB], F32, tag="uv")
        v_ps = psum.tile([128, NB], F32, tag="uv")
        for k in range(KC):
            nc.tensor.matmul(u4_ps, v1r_sb[:, k, :], x1t[:, k, :],
                             start=(k == 0), stop=(k == KC - 1))
        for k in range(KC):
            nc.tensor.matmul(v_ps, v2_sb[:, k, :], x2t[:, k, :],
                             start=(k == 0), stop=(k == KC - 1))

        v_sb = work.tile([128, NB], F16, tag="vsb")
        if CONFIG.get("u4_direct", False):
            # x3 mult reads u4 straight from PSUM (two-PSUM-operand DVE op)
            u4_sb = u4_ps
            nc.vector.tensor_copy(v_sb, v_ps)
        elif CONFIG.get("copies", "dve") == "dve":
            # keep ACT pure-Relu: function switches force ~1.3us ACT
            # function-table reloads that the cost model doesn't charge
            u4_sb = work.tile([128, NB], F16, tag="u4")
            nc.vector.tensor_copy(v_sb, v_ps)
            nc.vector.tensor_copy(u4_sb, u4_ps)
        else:
            u4_sb = work.tile([128, NB], F16, tag="u4")
            nc.vector.tensor_copy(u4_sb, u4_ps)  # DVE, parallel with ACT
            nc.scalar.activation(v_sb, v_ps, AF.Copy)

        x3 = work.tile([128, KC, NB], F16, tag="x3")
        for k in range(KC):
            if variant == "nox3":
                nc.vector.tensor_copy(x3[:, k, :], u4_sb)
                continue
            if CONFIG.get("vk", "mm") == "dma":
                # V_k via SBUF->SBUF broadcast DMA (stride-0 partition
                # reads) instead of a PE selector matmul.
                vk_sb = work.tile([128, NB], F16, tag="vkb", bufs=3)
                src = v_sb[4 * k:4 * k + 4, None, :].to_broadcast([4, 32, NB])
                dst = vk_sb.rearrange("(c q) n -> c q n", q=32)
                eng = getattr(nc, CONFIG.get("vk_dma_engine", "gpsimd"))
                eng.dma_start(dst, src)
                nc.vector.tensor_mul(x3[:, k, :], vk_sb, u4_sb)
            else:
                vk_ps = psum.tile([128, NB], F32,
                                  tag=CONFIG.get("vk_tag", "vk"),
                                  bufs=CONFIG.get("vk_bufs", 2))
                nc.tensor.matmul(vk_ps, sel_sb[0:32, k, :], v_sb[0:32, :],
                                 start=True, stop=True)
                nc.vector.tensor_mul(x3[:, k, :], vk_ps, u4_sb)
        return x3

    def tail(i, x3):
        bs = slice(i * NB, (i + 1) * NB)
        h1 = work.tile([128, 4, NB], F16, tag="h1")
        if CONFIG["w1_order"] == "k":
            # k-outer/m-inner: W1 matmuls for x3[k] start as soon as the
            # DVE produces that slice; 4 banks accumulate m=0..3.
            h1_pss = [psum.tile([128, NB], F32, tag="h1ps",
                                bufs=CONFIG["h1_bufs"],
                                name=f"h1ps{m}") for m in range(4)]
            for k in range(KC):
                for m in range(4):
                    nc.tensor.matmul(
                        h1_pss[m], w1_sb[:, k, m * 128:(m + 1) * 128],
                        x3[:, k, :],
                        start=(k == 0), stop=(k == KC - 1))
            for m in range(4):
                nc.scalar.activation(h1[:, m, :], h1_pss[m], AF.Relu,
                                     bias=b1_sb[:, m:m + 1])
        else:
            for m in range(4):
                h1_ps = psum.tile([128, NB], F32, tag="h1ps",
                                  bufs=CONFIG["h1_bufs"])
                for k in range(KC):
                    nc.tensor.matmul(
                        h1_ps, w1_sb[:, k, m * 128:(m + 1) * 128],
                        x3[:, k, :],
                        start=(k == 0), stop=(k == KC - 1))
                nc.scalar.activation(h1[:, m, :], h1_ps, AF.Relu,
                                     bias=b1_sb[:, m:m + 1])

        if variant == "w1only":
            o_sb = work.tile([1, NB], F32, tag="osb")
            nc.vector.tensor_copy(o_sb, h1[0:1, 0, :])
            nc.gpsimd.dma_start(out_d[bs, :].rearrange("a b -> b a"), o_sb)
            return

        tt, tb = CONFIG["tail_tag"], CONFIG["tail_bufs"]
        h2_ps = psum.tile([128, NB], F32, tag=tt, bufs=tb)
        for k in range(4):
            nc.tensor.matmul(h2_ps, w2_sb[:, k, :], h1[:, k, :],
                             start=(k == 0), stop=(k == 3))
        h2 = work.tile([128, NB], F16, tag="h2")
        nc.scalar.activation(h2, h2_ps, AF.Relu, bias=b2_sb[:, 0:1])

        h3_ps = psum.tile([128, NB], F32, tag=tt, bufs=tb)
        nc.tensor.matmul(h3_ps, w3_sb, h2, start=True, stop=True)
        h3 = work.tile([128, NB], F16, tag="h3")
        nc.scalar.activation(h3, h3_ps, AF.Relu, bias=b3_sb[:, 0:1])

        o_ps = psum.tile([1, NB], F32, tag=tt, bufs=tb)
        nc.tensor.matmul(o_ps, w4_sb, h3, start=True, stop=True)
        o_sb = work.tile([1, NB], F32, tag="osb")
        if CONFIG.get("copies", "dve") == "dve":
            nc.vector.tensor_scalar_add(o_sb, o_ps, b4_val)
        else:
            nc.scalar.activation(o_sb, o_ps, AF.Copy, bias=b4_val)

        # store via SWDGE: keeps the SP HWDGE ring pure input loads
        nc.gpsimd.dma_start(out_d[bs, :].rearrange("a b -> b a"), o_sb)

    if variant == "dmaonly":
        for i in range(NCHUNK):
            head(i)
        return

    if CONFIG.get("pipeline", True):
        prev = None
        for i in range(NCHUNK):
            x3 = head(i)
            if prev is not None:
                tail(i - 1, prev)
            prev = x3
        tail(NCHUNK - 1, prev)
    else:
        for i in range(NCHUNK):
            tail(i, head(i))


def _prep_consts(var_1, var_2, W1, b1, W2, b2, W3, b3, W4):
    f16 = np.float16
    v1r = np.tile(var_1.T, (1, 4))  # [1024, 128] -> U4 = tile(u, 4)
    v1r_l = np.ascontiguousarray(
        v1r.reshape(KC, 128, 128).transpose(1, 0, 2)).astype(f16)
    v2r = np.tile(var_2, (1, 4))  # [1024, 128] -> V4 = tile(v, 4)
    v2_l = np.ascontiguousarray(
        v2r.reshape(KC, 128, 128).transpose(1, 0, 2)).astype(f16)

    # selector for the V_k partition-broadcast matmuls (K=32)
    sel = np.zeros((128, KC, 128), f16)
    p = np.arange(128)
    for k in range(KC):
        sel[4 * k + p // 32, k, p] = 1.0

    w1_l = np.ascontiguousarray(
        W1.reshape(KC, 128, 512).transpose(1, 0, 2)).astype(f16)
    w2_l = np.ascontiguousarray(
        W2.reshape(4, 128, 128).transpose(1, 0, 2)).astype(f16)
    w3_l = np.zeros((128, 128), f16)
    w3_l[:, :R] = W3.astype(f16)
    w4_l = np.zeros((128, 1), f16)
    w4_l[:R, :] = W4.astype(f16)

    b1_l = np.ascontiguousarray(b1.reshape(4, 128).T).astype(np.float32)
    b2_l = b2.reshape(128, 1).astype(np.float32)
    b3_l = np.zeros((128, 1), np.float32)
    b3_l[:R, 0] = b3
    return dict(v1r=v1r_l, v2l=v2_l, sel=sel, w1l=w1_l, w2l=w2_l,
                w3l=w3_l, w4l=w4_l, b1l=b1_l, b2l=b2_l, b3l=b3_l)


def make_in_maps(**inputs):
    """Shard inputs across cores; returns (in_maps, b4_val)."""
    x1 = np.asarray(inputs["x1"], np.float32)
    x2 = np.asarray(inputs["x2"], np.float32)
    consts = _prep_consts(
        *[np.asarray(inputs[k], np.float32) for k in
          ("var_1", "var_2", "W1", "b1", "W2", "b2", "W3", "b3", "W4")])
    x1h = x1.astype(np.float16)
    x2h = x2.astype(np.float16)

    def shard_t(xh, c):
        # [BL, D] -> [NCHUNK, 128, KC, NB]: [i, p, k, b] = x[i*NB+b, 128k+p]
        xs = xh[c * BL:(c + 1) * BL]  # [BL, D]
        x4 = xs.reshape(NCHUNK, NB, KC, 128)  # [i, b, k, p]
        return np.ascontiguousarray(x4.transpose(0, 3, 2, 1))

    in_maps = []
    for c in range(NCORES):
        m = dict(consts)
        m["x1"] = shard_t(x1h, c)
        m["x2"] = shard_t(x2h, c)
        in_maps.append(m)
    return in_maps, float(np.asarray(inputs["b4"]).reshape(-1)[0])


def run(trace=False, **inputs):
    in_maps, b4_val = make_in_maps(**inputs)
    nc = _build(b4_val)
    res = bass_utils.run_bass_kernel_spmd(
        nc, in_maps, core_ids=list(range(NCORES)), trace=trace)
    out = np.concatenate([r["out"] for r in res.results], axis=0)
    return out.astype(np.float32), res


def kernel(**inputs):
    out, _ = run(trace=False, **inputs)
    return out
